# revision 10
# baseline (speedup 1.0000x reference)
"""Trainium2 Bass kernel for ConditionalNeuralNetwork (MoE-style routed MLP).

Strategy (expert-parallel over combos, data-parallel within a combo):
  - Host computes combo idx = 2*flags[:,0] + flags[:,1] per row, groups rows
    by combo, and splits each combo's rows across 2 of the 8 cores.
  - Each core runs a dense MLP 256 -> 1024 -> 1024 -> 512 -> 256 -> 1 on its
    rows with only ITS head's weights (relu between layers; final bias +
    sigmoid applied on the host from the fp32 logits).
  - All matmuls run in fp8(e4m3) with perf_mode=DoubleRow (2 fp8 weights per
    PE cell = 2 MACs/cell/cycle). Accumulation is fp32 in PSUM; epilogues
    (bias+relu) alternate ACT/DVE and write fp8 activations directly.
  - Loop order is weight-stationary: for each (m-tile, k-pair) the stationary
    weights are loaded once and all row-chunks stream through; redundant
    LDWEIGHTS of the same weights are deleted post-build.
  - Prologue: tiny memset + a couple of N=128 warmup matmuls start the HAM
    clock ramp right after the SPMD start barrier while the first weight/x
    DMAs (split across the three DMA-capable queues) land.
  - Tail: HL2+HL3 run fused per chunk-pair so early pairs' logits DMA out
    while later pairs still compute; the last (tiny) pair leaves only ~1us
    after the final matmul.
  - Host scatters per-core outputs back to original row order and applies
    sigmoid(logit + b3).
"""

import os
import sys

import ml_dtypes
import numpy as np

for _p in ("/opt/trn_rl_repo", "/root/.axon_site/_ro/trn_rl_repo"):
    if os.path.isdir(_p) and _p not in sys.path:
        sys.path.append(_p)

import concourse.bacc as bacc
import concourse.bass as bass
import concourse.tile as tile
from concourse import mybir
from concourse.bass import MemorySpace
from concourse.bass_utils import run_bass_kernel_spmd

F32 = mybir.dt.float32
BF16 = mybir.dt.bfloat16
F8 = mybir.dt.float8e4
AF = mybir.ActivationFunctionType
DR = mybir.MatmulPerfMode.DoubleRow
NPBF16 = ml_dtypes.bfloat16
NPF8 = ml_dtypes.float8_e4m3  # TRN fp8e4: max +-240, RNE

B, D_IN = 16384, 256
S1, S2 = 1024, 1024
H1, H2 = 512, 256
C = 4
NCORES = 8
CAP = 2080  # rows per core; seed-0 max shard is exactly 2080
WARMUP_MMS = int(os.environ.get("K_WARMUP", "12"))

_nc_cache = {}
_last_results = None


def _chunks(cap):
    """Row chunks of <=512 (PSUM bank): [(off, n), ...]."""
    assert cap % 32 == 0 and cap <= 2560
    out = []
    off = 0
    while off < cap:
        n = min(512, cap - off)
        out.append((off, n))
        off += n
    return out


def _dedup_ldweights(nc):
    """Remove back-to-back InstLdweights that reload identical weights.

    The rust add_instruction splits every matmul into LDWEIGHTS + MATMUL.
    With the weight-stationary loop order most loads are redundant; the PE
    keeps the stationary operand between matmuls. Any waits on a removed
    LDWEIGHTS are merged into the instruction that followed it.
    """
    removed = kept = 0
    for f in nc.m.functions:
        for blk in f.blocks:
            insts = list(blk.instructions)
            new = []
            last_key = None
            pending_waits = []
            for inst in insts:
                nm = type(inst).__name__
                if nm == "InstLdweights":
                    key = (repr(inst.ins[0]), inst.perf_mode,
                           inst.tile_position, inst.is_transpose)
                    si = inst.sync_info
                    has_upd = bool(si is not None and si.on_update)
                    if key == last_key and not has_upd:
                        if si is not None and si.on_wait:
                            pending_waits.extend(si.on_wait)
                        removed += 1
                        continue
                    last_key = key
                    kept += 1
                elif nm == "InstMatmult":
                    if pending_waits:
                        si = inst.sync_info
                        if si is None:
                            inst.sync_info = mybir.SyncInfo(
                                on_wait=list(pending_waits), on_update=[])
                        else:
                            si.on_wait = list(si.on_wait) + pending_waits
                        pending_waits = []
                # Other instruction kinds run on non-PE engines (or are
                # semaphore ops) and do not disturb the PE weight array, so
                # the cached key stays valid across them.
                new.append(inst)
            assert not pending_waits
            blk.instructions[:] = new
    return removed, kept


def _build(cap=CAP):
    """Build the single-core MLP program (SPMD across 8 cores)."""
    nc = bacc.Bacc("TRN2", target_bir_lowering=False, debug=False)

    def din(name, shape, dt=F8):
        return nc.dram_tensor(name, list(shape), dt, kind="ExternalInput").ap()

    chs = _chunks(cap)
    ncks = len(chs)
    # pairs of chunks sharing one 2-bank psum tile
    prs = [tuple(range(i, min(i + 2, ncks))) for i in range(0, ncks, 2)]

    xT = din("xT", [128, 2, cap])            # x rows, k-major
    w1 = din("w1", [128, 2, S1])
    w2 = din("w2", [128, 8, S2])
    hw1 = din("hw1", [128, 8, H1])
    hw2 = din("hw2", [128, 4, H2])
    hw3 = din("hw3", [128, 2, 16])           # w3 in col 0, zero-padded
    # biases packed into one tensor: [b1(8) | b2(8) | hb1(4) | hb2(2) | pad]
    cst = din("consts", [128, 23], F32)
    out = nc.dram_tensor("out", [1, cap], F32, kind="ExternalOutput").ap()

    ALU = mybir.AluOpType

    with tile.TileContext(nc) as tc:
        with tc.tile_pool(name="weights", bufs=1) as wp, \
             tc.tile_pool(name="acts", bufs=1) as ap_, \
             tc.tile_pool(name="outs", bufs=4) as op, \
             tc.tile_pool(name="psum", bufs=4, space=MemorySpace.PSUM) as pp:

            w1s = wp.tile([128, 2, S1], F8, tag="w1s")
            w2s = wp.tile([128, 8, S2], F8, tag="w2s")
            hw1s = wp.tile([128, 8, H1], F8, tag="hw1s")
            hw2s = wp.tile([128, 4, H2], F8, tag="hw2s")
            hw3s = wp.tile([128, 2, 16], F8, tag="hw3s")
            csts = wp.tile([128, 23], F32, tag="csts")
            b1s = csts[:, 0:8]
            b2s = csts[:, 8:16]
            hb1s = csts[:, 16:20]
            hb2s = csts[:, 20:22]

            # activations stay resident for all chunks (weight-stationary);
            # [128, ktiles, chunk, 512] with the ragged tail chunk padded
            xts = ap_.tile([128, 2, cap], F8, tag="xts")
            h1s = ap_.tile([128, 8, ncks, 512], F8, tag="h1s")
            h2s = ap_.tile([128, 8, ncks, 512], F8, tag="h2s")
            a1s = ap_.tile([128, 4, ncks, 512], F8, tag="a1s")
            a2s = ap_.tile([128, 2, ncks, 512], F8, tag="a2s")

            # PE warm-up: dependency-free matmuls ramp the HAM clock and
            # fill the ~8us DMA-completion window before real matmuls.
            if WARMUP_MMS:
                wut = wp.tile([128, 512], BF16, tag="wut")
                nc.vector.memset(wut[:], 0.0)
                wups = pp.tile([128, 2, 512], F32, tag="ps")
                for _ in range(WARMUP_MMS):
                    nc.tensor.matmul(wups[:, 0, :], wut[:, 0:128],
                                     wut[:, :], start=True, stop=True)

            # DMAs across the three DMA-capable queues (sync/scalar/gpsimd),
            # ordered so completion semaphores fire in the order the compute
            # stream consumes them (w1+x0 first, then x1..x3/csts staggered,
            # then w2/hw*).  Transfers only start ~4us after their
            # descriptor (ring round-trip), so the cascade matters.
            nc.sync.dma_start(out=w1s[:, 0, :], in_=w1[:, 0, :])
            nc.scalar.dma_start(out=xts[:, :, 0:512], in_=xT[:, :, 0:512])
            nc.gpsimd.dma_start(out=w1s[:, 1, :], in_=w1[:, 1, :])
            nc.sync.dma_start(out=csts[:], in_=cst[:])
            nc.sync.dma_start(out=xts[:, :, 512:1024], in_=xT[:, :, 512:1024])
            nc.scalar.dma_start(out=xts[:, :, 1024:1536],
                                in_=xT[:, :, 1024:1536])
            nc.gpsimd.dma_start(out=xts[:, :, 1536:cap], in_=xT[:, :, 1536:cap])
            for k in range(8):
                nc.sync.dma_start(out=w2s[:, k, :], in_=w2[:, k, :])
            for k in range(8):
                nc.gpsimd.dma_start(out=hw1s[:, k, :], in_=hw1[:, k, :])
            nc.gpsimd.dma_start(out=hw2s[:], in_=hw2[:])
            nc.gpsimd.dma_start(out=hw3s[:], in_=hw3[:])

            # Bias+relu epilogue, alternating ACT / DVE.
            epi_n = [0]

            def epilogue(dst, src, bias_ap):
                if epi_n[0] % 2 == 0:
                    nc.scalar.activation(dst, src, AF.Relu, bias=bias_ap)
                else:
                    nc.vector.tensor_scalar(
                        dst, src, bias_ap, 0.0, ALU.add, ALU.max)
                epi_n[0] += 1

            def layer(rhs, ktiles, wt, nm, bias, dst):
                """dst[:,m,c,:] = relu(sum_k wt[:,k,m]T @ rhs(kp,c) + b)"""
                npair = ktiles // 2
                for m in range(nm):
                    tt = [pp.tile([128, 2, 512], F32, tag="ps", name=f"t{j}")
                          for j in range(len(prs))]
                    for kp in range(npair):
                        lhs = wt[:, 2 * kp:2 * kp + 2,
                                 m * 128:(m + 1) * 128]
                        for c, (off, n) in enumerate(chs):
                            nc.tensor.matmul(
                                tt[c // 2][:, c % 2, :n], lhs, rhs(kp, c),
                                start=(kp == 0), stop=(kp == npair - 1),
                                perf_mode=DR)
                    for j, pr in enumerate(prs):
                        if len(pr) == 2 and chs[pr[0]][1] == chs[pr[1]][1]:
                            epilogue(dst[:, m, pr[0]:pr[0] + 2,
                                         :chs[pr[0]][1]],
                                     tt[j][:, :, :chs[pr[0]][1]],
                                     bias[:, m:m + 1])
                        else:
                            for c2, c in enumerate(pr):
                                epilogue(dst[:, m, c, :chs[c][1]],
                                         tt[j][:, c2, :chs[c][1]],
                                         bias[:, m:m + 1])

            def hslice(t):
                return lambda kp, c: t[:, 2 * kp:2 * kp + 2, c, :chs[c][1]]

            layer(lambda kp, c: xts[:, :, chs[c][0]:chs[c][0] + chs[c][1]],
                  2, w1s, 8, b1s, h1s)                 # L1: 256 -> 1024
            layer(hslice(h1s), 8, w2s, 8, b2s, h2s)    # L2: 1024 -> 1024
            layer(hslice(h2s), 8, hw1s, 4, hb1s, a1s)  # HL1: 1024 -> 512

            # HL2 (512 -> 256) + HL3 (256 -> 1) software-pipelined per chunk
            # pair: HL3 of pair j is emitted after HL2 of pair j+1 so the
            # in-order PE never stalls on pair-j epilogues, and early pairs'
            # logits DMA out while later pairs still compute.  The ragged
            # tail pair comes last, leaving a tiny post-matmul tail.
            cp_n = [0]

            def hl2_pair(j):
                pr = prs[j]
                for m in range(2):
                    tt = pp.tile([128, 2, 512], F32, tag="ps",
                                 name=f"t2_{j}_{m}")
                    for kp in range(2):
                        lhs = hw2s[:, 2 * kp:2 * kp + 2,
                                   m * 128:(m + 1) * 128]
                        for c2, c in enumerate(pr):
                            nc.tensor.matmul(
                                tt[:, c2, :chs[c][1]], lhs,
                                a1s[:, 2 * kp:2 * kp + 2, c, :chs[c][1]],
                                start=(kp == 0), stop=(kp == 1),
                                perf_mode=DR)
                    if len(pr) == 2 and chs[pr[0]][1] == chs[pr[1]][1]:
                        epilogue(a2s[:, m, pr[0]:pr[0] + 2, :chs[pr[0]][1]],
                                 tt[:, :, :chs[pr[0]][1]], hb2s[:, m:m + 1])
                    else:
                        for c2, c in enumerate(pr):
                            epilogue(a2s[:, m, c, :chs[c][1]],
                                     tt[:, c2, :chs[c][1]], hb2s[:, m:m + 1])

            # logits land in one contiguous SBUF strip; one out-DMA per pair
            ots = op.tile([1, cap], F32, tag="ots")

            def hl3_pair(j):
                # HL3: 256 -> 1 logits (M=1); fp32 psum -> SBUF copies split
                # across ACT/DVE per chunk, one out-DMA per pair on sync;
                # bias+sigmoid happen host-side.
                pr = prs[j]
                psl = pp.tile([128, 2, 512], F32, tag="ps", name=f"t3_{j}")
                for c2, c in enumerate(pr):
                    nc.tensor.matmul(psl[0:1, c2, :chs[c][1]],
                                     hw3s[:, :, 0:1],
                                     a2s[:, :, c, :chs[c][1]],
                                     start=True, stop=True, perf_mode=DR)
                for c2, c in enumerate(pr):
                    off, n = chs[c]
                    if cp_n[0] % 2 == 0:
                        nc.scalar.activation(ots[:, off:off + n],
                                             psl[0:1, c2, :n], AF.Copy)
                    else:
                        nc.vector.tensor_scalar_add(ots[:, off:off + n],
                                                    psl[0:1, c2, :n], 0.0)
                    cp_n[0] += 1
                lo, hi = chs[pr[0]][0], chs[pr[-1]][0] + chs[pr[-1]][1]
                nc.sync.dma_start(out=out[:, lo:hi], in_=ots[:, lo:hi])

            seq = []
            for j in range(len(prs)):
                seq.append(("hl2", j))
                if j >= 1:
                    seq.append(("hl3", j - 1))
            seq.append(("hl3", len(prs) - 1))
            for kind, j in seq:
                (hl2_pair if kind == "hl2" else hl3_pair)(j)

    _dedup_ldweights(nc)
    nc.compile()
    return nc


def _get_nc(cap=CAP):
    if cap not in _nc_cache:
        _nc_cache[cap] = _build(cap)
    return _nc_cache[cap]


def _q8(v):
    return np.clip(v, -240.0, 240.0).astype(NPF8)


def _tile_k(w, ktiles):
    """[K, M] -> [128, ktiles, M] fp8 with K = ktiles*128, K idx = k*128+p."""
    k, m = w.shape
    assert k == ktiles * 128
    return np.ascontiguousarray(
        _q8(w.reshape(ktiles, 128, m).transpose(1, 0, 2)))


def _tile_b(b):
    """[M] -> [128, M/128] f32; column m holds bias for m-tile m."""
    m = b.shape[0]
    return np.ascontiguousarray(b.reshape(m // 128, 128).T.astype(np.float32))


def _make_in_maps(inputs):
    x = np.asarray(inputs["x"], dtype=np.float32)
    ff = np.asarray(inputs["feature_flags"]).astype(np.int64)
    idx = ff[:, 0] * 2 + ff[:, 1]

    W1 = np.asarray(inputs["W1"], np.float32)
    b1 = np.asarray(inputs["b1"], np.float32)
    W2 = np.asarray(inputs["W2"], np.float32)
    b2 = np.asarray(inputs["b2"], np.float32)
    HW1 = np.asarray(inputs["HW1"], np.float32)
    Hb1 = np.asarray(inputs["Hb1"], np.float32)
    HW2 = np.asarray(inputs["HW2"], np.float32)
    Hb2 = np.asarray(inputs["Hb2"], np.float32)
    HW3 = np.asarray(inputs["HW3"], np.float32)

    # Row assignment: combo c -> cores 2c, 2c+1.
    row_sets = []
    for c in range(C):
        rows = np.nonzero(idx == c)[0]
        h = (len(rows) + 1) // 2
        row_sets.append(rows[:h])
        row_sets.append(rows[h:])
    max_shard = max(len(r) for r in row_sets)
    cap = max(CAP, -(-max_shard // 32) * 32)

    w1t = _tile_k(W1, 2)
    w2t = _tile_k(W2, 8)
    hw1t = [_tile_k(HW1[c], 8) for c in range(C)]
    hw2t = [_tile_k(HW2[c], 4) for c in range(C)]
    hw3t = []
    for c in range(C):
        t = np.zeros((128, 2, 16), NPF8)
        t[:, :, 0] = _q8(HW3[c][:, 0].reshape(2, 128).T)
        hw3t.append(t)
    cstt = []
    for c in range(C):
        cst = np.zeros((128, 23), np.float32)
        cst[:, 0:8] = _tile_b(b1)
        cst[:, 8:16] = _tile_b(b2)
        cst[:, 16:20] = _tile_b(Hb1[c])
        cst[:, 20:22] = _tile_b(Hb2[c])
        cstt.append(cst)

    in_maps = []
    for d, rows in enumerate(row_sets):
        c = d // 2
        n = len(rows)
        xt = np.zeros((128, 2, cap), NPF8)
        if n:
            xt[:, :, :n] = _q8(x[rows].T.reshape(2, 128, n).transpose(
                1, 0, 2))
        in_maps.append({
            "xT": xt,
            "w1": w1t, "w2": w2t,
            "hw1": hw1t[c], "hw2": hw2t[c], "hw3": hw3t[c],
            "consts": cstt[c],
        })

    return in_maps, row_sets, cap


def kernel(**inputs):
    global _last_results
    in_maps, row_sets, cap = _make_in_maps(inputs)
    nc = _get_nc(cap)
    res = run_bass_kernel_spmd(nc, in_maps, core_ids=list(range(NCORES)))
    _last_results = res

    Hb3 = np.asarray(inputs["Hb3"], np.float64)
    out = np.empty(B, np.float32)
    for d, rows in enumerate(row_sets):
        if len(rows):
            logits = res.results[d]["out"][0, :len(rows)].astype(np.float64)
            logits += Hb3[d // 2, 0]
            out[rows] = (1.0 / (1.0 + np.exp(-logits))).astype(np.float32)
    return out


# revision 13
# speedup vs baseline: 1.0154x; 1.0154x over previous
"""Trainium2 Bass kernel for ConditionalNeuralNetwork (MoE-style routed MLP).

Strategy (expert-parallel over combos, data-parallel within a combo):
  - Host computes combo idx = 2*flags[:,0] + flags[:,1] per row, groups rows
    by combo, and splits each combo's rows across 2 of the 8 cores.
  - Each core runs a dense MLP 256 -> 1024 -> 1024 -> 512 -> 256 -> 1 on its
    rows with only ITS head's weights (relu between layers; final bias +
    sigmoid applied on the host from the fp32 logits).
  - All matmuls run in fp8(e4m3) with perf_mode=DoubleRow (2 fp8 weights per
    PE cell = 2 MACs/cell/cycle). Accumulation is fp32 in PSUM; epilogues
    (bias+relu) alternate ACT/DVE and write fp8 activations directly.
  - Loop order is weight-stationary: for each (m-tile, k-pair) the stationary
    weights are loaded once and all row-chunks stream through; redundant
    LDWEIGHTS of the same weights are deleted post-build.
  - Prologue: tiny memset + a couple of N=128 warmup matmuls start the HAM
    clock ramp right after the SPMD start barrier while the first weight/x
    DMAs (split across the three DMA-capable queues) land.
  - Tail: HL2+HL3 run fused per chunk-pair so early pairs' logits DMA out
    while later pairs still compute; the last (tiny) pair leaves only ~1us
    after the final matmul.
  - Host scatters per-core outputs back to original row order and applies
    sigmoid(logit + b3).
"""

import os
import sys

import ml_dtypes
import numpy as np

for _p in ("/opt/trn_rl_repo", "/root/.axon_site/_ro/trn_rl_repo"):
    if os.path.isdir(_p) and _p not in sys.path:
        sys.path.append(_p)

import concourse.bacc as bacc
import concourse.bass as bass
import concourse.tile as tile
from concourse import mybir
from concourse.bass import MemorySpace
from concourse.bass_utils import run_bass_kernel_spmd

F32 = mybir.dt.float32
BF16 = mybir.dt.bfloat16
F8 = mybir.dt.float8e4
AF = mybir.ActivationFunctionType
DR = mybir.MatmulPerfMode.DoubleRow
NPBF16 = ml_dtypes.bfloat16
NPF8 = ml_dtypes.float8_e4m3  # TRN fp8e4: max +-240, RNE

B, D_IN = 16384, 256
S1, S2 = 1024, 1024
H1, H2 = 512, 256
C = 4
NCORES = 8
CAP = 2080  # rows per core; seed-0 max shard is exactly 2080
W512 = int(os.environ.get("K_W512", "8"))
W256 = int(os.environ.get("K_W256", "20"))

_nc_cache = {}
_last_results = None


def _chunks(cap):
    """Row chunks of <=512 (PSUM bank): [(off, n), ...]."""
    assert cap % 32 == 0 and cap <= 2560
    out = []
    off = 0
    while off < cap:
        n = min(512, cap - off)
        out.append((off, n))
        off += n
    return out


def _dedup_ldweights(nc):
    """Remove back-to-back InstLdweights that reload identical weights.

    The rust add_instruction splits every matmul into LDWEIGHTS + MATMUL.
    With the weight-stationary loop order most loads are redundant; the PE
    keeps the stationary operand between matmuls. Any waits on a removed
    LDWEIGHTS are merged into the instruction that followed it.
    """
    removed = kept = 0
    for f in nc.m.functions:
        for blk in f.blocks:
            insts = list(blk.instructions)
            new = []
            last_key = None
            pending_waits = []
            for inst in insts:
                nm = type(inst).__name__
                if nm == "InstLdweights":
                    key = (repr(inst.ins[0]), inst.perf_mode,
                           inst.tile_position, inst.is_transpose)
                    si = inst.sync_info
                    has_upd = bool(si is not None and si.on_update)
                    if key == last_key and not has_upd:
                        if si is not None and si.on_wait:
                            pending_waits.extend(si.on_wait)
                        removed += 1
                        continue
                    last_key = key
                    kept += 1
                elif nm == "InstMatmult":
                    if pending_waits:
                        si = inst.sync_info
                        if si is None:
                            inst.sync_info = mybir.SyncInfo(
                                on_wait=list(pending_waits), on_update=[])
                        else:
                            si.on_wait = list(si.on_wait) + pending_waits
                        pending_waits = []
                # Other instruction kinds run on non-PE engines (or are
                # semaphore ops) and do not disturb the PE weight array, so
                # the cached key stays valid across them.
                new.append(inst)
            assert not pending_waits
            blk.instructions[:] = new
    return removed, kept


def _build(cap=CAP):
    """Build the single-core MLP program (SPMD across 8 cores)."""
    nc = bacc.Bacc("TRN2", target_bir_lowering=False, debug=False)

    def din(name, shape, dt=F8):
        return nc.dram_tensor(name, list(shape), dt, kind="ExternalInput").ap()

    chs = _chunks(cap)
    ncks = len(chs)
    # pairs of chunks sharing one 2-bank psum tile
    prs = [tuple(range(i, min(i + 2, ncks))) for i in range(0, ncks, 2)]

    xT = din("xT", [128, 2, cap])            # x rows, k-major
    w1 = din("w1", [128, 2, S1])
    w2 = din("w2", [128, 8, S2])
    hw1 = din("hw1", [128, 8, H1])
    hw2 = din("hw2", [128, 4, H2])
    hw3 = din("hw3", [128, 2, 16])           # w3 in col 0, zero-padded
    # biases packed into one tensor: [b1(8) | b2(8) | hb1(4) | hb2(2) | pad]
    cst = din("consts", [128, 23], F32)
    out = nc.dram_tensor("out", [1, cap], F32, kind="ExternalOutput").ap()

    ALU = mybir.AluOpType

    with tile.TileContext(nc) as tc:
        with tc.tile_pool(name="weights", bufs=1) as wp, \
             tc.tile_pool(name="acts", bufs=1) as ap_, \
             tc.tile_pool(name="outs", bufs=4) as op, \
             tc.tile_pool(name="psum", bufs=4, space=MemorySpace.PSUM) as pp:

            w1s = wp.tile([128, 2, S1], F8, tag="w1s")
            w2s = wp.tile([128, 8, S2], F8, tag="w2s")
            hw1s = wp.tile([128, 8, H1], F8, tag="hw1s")
            hw2s = wp.tile([128, 4, H2], F8, tag="hw2s")
            hw3s = wp.tile([128, 2, 16], F8, tag="hw3s")
            csts = wp.tile([128, 23], F32, tag="csts")
            b1s = csts[:, 0:8]
            b2s = csts[:, 8:16]
            hb1s = csts[:, 16:20]
            hb2s = csts[:, 20:22]

            # activations stay resident for all chunks (weight-stationary);
            # [128, ktiles, chunk, 512] with the ragged tail chunk padded
            xts = ap_.tile([128, 2, cap], F8, tag="xts")
            h1s = ap_.tile([128, 8, ncks, 512], F8, tag="h1s")
            h2s = ap_.tile([128, 8, ncks, 512], F8, tag="h2s")
            a1s = ap_.tile([128, 4, ncks, 512], F8, tag="a1s")
            a2s = ap_.tile([128, 2, ncks, 512], F8, tag="a2s")

            # PE warm-up: dependency-free matmuls ramp the HAM clock and
            # fill the ~11us DMA-completion window before real matmuls.
            # Two phases: big N=512 MMs for the bulk, then N=256 MMs so the
            # handoff to the (DMA-gated) first real matmul is fine-grained —
            # a short overshoot is far cheaper than a PE gap, which would
            # re-throttle HAM for ~7us.
            if W512 or W256:
                wut = wp.tile([128, 512], BF16, tag="wut")
                nc.vector.memset(wut[:], 0.0)
                wups = pp.tile([128, 2, 512], F32, tag="ps")
                for _ in range(W512):
                    nc.tensor.matmul(wups[:, 0, :], wut[:, 0:128],
                                     wut[:, :], start=True, stop=True)
                for _ in range(W256):
                    nc.tensor.matmul(wups[:, 0, 0:256], wut[:, 0:128],
                                     wut[:, 0:256], start=True, stop=True)

            # DMAs across the three DMA-capable queues (sync/scalar/gpsimd),
            # ordered so completion semaphores fire in the order the compute
            # stream consumes them (w1+x0 first, then x1..x3/csts staggered,
            # then w2/hw*).  Transfers only start ~4us after their
            # descriptor (ring round-trip), so the cascade matters.
            nc.sync.dma_start(out=w1s[:, 0, :], in_=w1[:, 0, :])
            nc.scalar.dma_start(out=xts[:, :, 0:512], in_=xT[:, :, 0:512])
            nc.gpsimd.dma_start(out=w1s[:, 1, :], in_=w1[:, 1, :])
            nc.sync.dma_start(out=csts[:], in_=cst[:])
            nc.sync.dma_start(out=xts[:, :, 512:1024], in_=xT[:, :, 512:1024])
            nc.scalar.dma_start(out=xts[:, :, 1024:1536],
                                in_=xT[:, :, 1024:1536])
            nc.gpsimd.dma_start(out=xts[:, :, 1536:cap], in_=xT[:, :, 1536:cap])
            for k in range(8):
                nc.sync.dma_start(out=w2s[:, k, :], in_=w2[:, k, :])
            for k in range(8):
                nc.gpsimd.dma_start(out=hw1s[:, k, :], in_=hw1[:, k, :])
            nc.gpsimd.dma_start(out=hw2s[:], in_=hw2[:])
            nc.gpsimd.dma_start(out=hw3s[:], in_=hw3[:])

            # Bias+relu epilogue, alternating ACT / DVE.
            epi_n = [0]

            def epilogue(dst, src, bias_ap):
                if epi_n[0] % 2 == 0:
                    nc.scalar.activation(dst, src, AF.Relu, bias=bias_ap)
                else:
                    nc.vector.tensor_scalar(
                        dst, src, bias_ap, 0.0, ALU.add, ALU.max)
                epi_n[0] += 1

            def layer(rhs, ktiles, wt, nm, bias, dst):
                """dst[:,m,c,:] = relu(sum_k wt[:,k,m]T @ rhs(kp,c) + b)"""
                npair = ktiles // 2
                for m in range(nm):
                    tt = [pp.tile([128, 2, 512], F32, tag="ps", name=f"t{j}")
                          for j in range(len(prs))]
                    for kp in range(npair):
                        lhs = wt[:, 2 * kp:2 * kp + 2,
                                 m * 128:(m + 1) * 128]
                        for c, (off, n) in enumerate(chs):
                            nc.tensor.matmul(
                                tt[c // 2][:, c % 2, :n], lhs, rhs(kp, c),
                                start=(kp == 0), stop=(kp == npair - 1),
                                perf_mode=DR)
                    for j, pr in enumerate(prs):
                        if len(pr) == 2 and chs[pr[0]][1] == chs[pr[1]][1]:
                            epilogue(dst[:, m, pr[0]:pr[0] + 2,
                                         :chs[pr[0]][1]],
                                     tt[j][:, :, :chs[pr[0]][1]],
                                     bias[:, m:m + 1])
                        else:
                            for c2, c in enumerate(pr):
                                epilogue(dst[:, m, c, :chs[c][1]],
                                         tt[j][:, c2, :chs[c][1]],
                                         bias[:, m:m + 1])

            def hslice(t):
                return lambda kp, c: t[:, 2 * kp:2 * kp + 2, c, :chs[c][1]]

            layer(lambda kp, c: xts[:, :, chs[c][0]:chs[c][0] + chs[c][1]],
                  2, w1s, 8, b1s, h1s)                 # L1: 256 -> 1024
            layer(hslice(h1s), 8, w2s, 8, b2s, h2s)    # L2: 1024 -> 1024
            layer(hslice(h2s), 8, hw1s, 4, hb1s, a1s)  # HL1: 1024 -> 512

            # HL2 (512 -> 256) + HL3 (256 -> 1) software-pipelined per chunk
            # pair: HL3 of pair j is emitted after HL2 of pair j+1 so the
            # in-order PE never stalls on pair-j epilogues, and early pairs'
            # logits DMA out while later pairs still compute.  The ragged
            # tail pair comes last, leaving a tiny post-matmul tail.
            cp_n = [0]

            def hl2_pair(j):
                pr = prs[j]
                for m in range(2):
                    tt = pp.tile([128, 2, 512], F32, tag="ps",
                                 name=f"t2_{j}_{m}")
                    for kp in range(2):
                        lhs = hw2s[:, 2 * kp:2 * kp + 2,
                                   m * 128:(m + 1) * 128]
                        for c2, c in enumerate(pr):
                            nc.tensor.matmul(
                                tt[:, c2, :chs[c][1]], lhs,
                                a1s[:, 2 * kp:2 * kp + 2, c, :chs[c][1]],
                                start=(kp == 0), stop=(kp == 1),
                                perf_mode=DR)
                    if len(pr) == 2 and chs[pr[0]][1] == chs[pr[1]][1]:
                        epilogue(a2s[:, m, pr[0]:pr[0] + 2, :chs[pr[0]][1]],
                                 tt[:, :, :chs[pr[0]][1]], hb2s[:, m:m + 1])
                    else:
                        for c2, c in enumerate(pr):
                            epilogue(a2s[:, m, c, :chs[c][1]],
                                     tt[:, c2, :chs[c][1]], hb2s[:, m:m + 1])

            # logits land in one contiguous SBUF strip; one out-DMA per pair
            ots = op.tile([1, cap], F32, tag="ots")

            def hl3_pair(j):
                # HL3: 256 -> 1 logits (M=1); fp32 psum -> SBUF copies split
                # across ACT/DVE per chunk, one out-DMA per pair on sync;
                # bias+sigmoid happen host-side.
                pr = prs[j]
                psl = pp.tile([128, 2, 512], F32, tag="ps", name=f"t3_{j}")
                for c2, c in enumerate(pr):
                    nc.tensor.matmul(psl[0:1, c2, :chs[c][1]],
                                     hw3s[:, :, 0:1],
                                     a2s[:, :, c, :chs[c][1]],
                                     start=True, stop=True, perf_mode=DR)
                for c2, c in enumerate(pr):
                    off, n = chs[c]
                    if cp_n[0] % 2 == 0:
                        nc.scalar.activation(ots[:, off:off + n],
                                             psl[0:1, c2, :n], AF.Copy)
                    else:
                        nc.vector.tensor_scalar_add(ots[:, off:off + n],
                                                    psl[0:1, c2, :n], 0.0)
                    cp_n[0] += 1
                # pair 0 DMAs from scalar; the last two pairs merge into one
                # sync DMA so only a single descriptor sits after the final
                # matmul.
                if j == 0:
                    lo, hi = chs[pr[0]][0], chs[pr[-1]][0] + chs[pr[-1]][1]
                    nc.scalar.dma_start(out=out[:, lo:hi], in_=ots[:, lo:hi])
                elif j == len(prs) - 1:
                    lo = chs[prs[1][0]][0]
                    hi = chs[pr[-1]][0] + chs[pr[-1]][1]
                    nc.sync.dma_start(out=out[:, lo:hi], in_=ots[:, lo:hi])

            seq = []
            for j in range(len(prs)):
                seq.append(("hl2", j))
                if j >= 1:
                    seq.append(("hl3", j - 1))
            seq.append(("hl3", len(prs) - 1))
            for kind, j in seq:
                (hl2_pair if kind == "hl2" else hl3_pair)(j)

    _dedup_ldweights(nc)
    nc.compile()
    return nc


def _get_nc(cap=CAP):
    if cap not in _nc_cache:
        _nc_cache[cap] = _build(cap)
    return _nc_cache[cap]


def _q8(v):
    return np.clip(v, -240.0, 240.0).astype(NPF8)


def _tile_k(w, ktiles):
    """[K, M] -> [128, ktiles, M] fp8 with K = ktiles*128, K idx = k*128+p."""
    k, m = w.shape
    assert k == ktiles * 128
    return np.ascontiguousarray(
        _q8(w.reshape(ktiles, 128, m).transpose(1, 0, 2)))


def _tile_b(b):
    """[M] -> [128, M/128] f32; column m holds bias for m-tile m."""
    m = b.shape[0]
    return np.ascontiguousarray(b.reshape(m // 128, 128).T.astype(np.float32))


def _make_in_maps(inputs):
    x = np.asarray(inputs["x"], dtype=np.float32)
    ff = np.asarray(inputs["feature_flags"]).astype(np.int64)
    idx = ff[:, 0] * 2 + ff[:, 1]

    W1 = np.asarray(inputs["W1"], np.float32)
    b1 = np.asarray(inputs["b1"], np.float32)
    W2 = np.asarray(inputs["W2"], np.float32)
    b2 = np.asarray(inputs["b2"], np.float32)
    HW1 = np.asarray(inputs["HW1"], np.float32)
    Hb1 = np.asarray(inputs["Hb1"], np.float32)
    HW2 = np.asarray(inputs["HW2"], np.float32)
    Hb2 = np.asarray(inputs["Hb2"], np.float32)
    HW3 = np.asarray(inputs["HW3"], np.float32)

    # Row assignment: combo c -> cores 2c, 2c+1.
    row_sets = []
    for c in range(C):
        rows = np.nonzero(idx == c)[0]
        h = (len(rows) + 1) // 2
        row_sets.append(rows[:h])
        row_sets.append(rows[h:])
    max_shard = max(len(r) for r in row_sets)
    cap = max(CAP, -(-max_shard // 32) * 32)

    w1t = _tile_k(W1, 2)
    w2t = _tile_k(W2, 8)
    hw1t = [_tile_k(HW1[c], 8) for c in range(C)]
    hw2t = [_tile_k(HW2[c], 4) for c in range(C)]
    hw3t = []
    for c in range(C):
        t = np.zeros((128, 2, 16), NPF8)
        t[:, :, 0] = _q8(HW3[c][:, 0].reshape(2, 128).T)
        hw3t.append(t)
    cstt = []
    for c in range(C):
        cst = np.zeros((128, 23), np.float32)
        cst[:, 0:8] = _tile_b(b1)
        cst[:, 8:16] = _tile_b(b2)
        cst[:, 16:20] = _tile_b(Hb1[c])
        cst[:, 20:22] = _tile_b(Hb2[c])
        cstt.append(cst)

    in_maps = []
    for d, rows in enumerate(row_sets):
        c = d // 2
        n = len(rows)
        xt = np.zeros((128, 2, cap), NPF8)
        if n:
            xt[:, :, :n] = _q8(x[rows].T.reshape(2, 128, n).transpose(
                1, 0, 2))
        in_maps.append({
            "xT": xt,
            "w1": w1t, "w2": w2t,
            "hw1": hw1t[c], "hw2": hw2t[c], "hw3": hw3t[c],
            "consts": cstt[c],
        })

    return in_maps, row_sets, cap


def kernel(**inputs):
    global _last_results
    in_maps, row_sets, cap = _make_in_maps(inputs)
    nc = _get_nc(cap)
    res = run_bass_kernel_spmd(nc, in_maps, core_ids=list(range(NCORES)))
    _last_results = res

    Hb3 = np.asarray(inputs["Hb3"], np.float64)
    out = np.empty(B, np.float32)
    for d, rows in enumerate(row_sets):
        if len(rows):
            logits = res.results[d]["out"][0, :len(rows)].astype(np.float64)
            logits += Hb3[d // 2, 0]
            out[rows] = (1.0 / (1.0 + np.exp(-logits))).astype(np.float32)
    return out


# revision 16
# speedup vs baseline: 1.0309x; 1.0153x over previous
"""Trainium2 Bass kernel for ConditionalNeuralNetwork (MoE-style routed MLP).

Strategy (expert-parallel over combos, data-parallel within a combo):
  - Host computes combo idx = 2*flags[:,0] + flags[:,1] per row, groups rows
    by combo, and splits each combo's rows across 2 of the 8 cores.
  - Each core runs a dense MLP 256 -> 1024 -> 1024 -> 512 -> 256 -> 1 on its
    rows with only ITS head's weights (relu between layers; final bias +
    sigmoid applied on the host from the fp32 logits).
  - All matmuls run in fp8(e4m3) with perf_mode=DoubleRow (2 fp8 weights per
    PE cell = 2 MACs/cell/cycle). Accumulation is fp32 in PSUM; epilogues
    (bias+relu) alternate ACT/DVE and write fp8 activations directly.
  - Loop order is weight-stationary: for each (m-tile, k-pair) the stationary
    weights are loaded once and all row-chunks stream through; redundant
    LDWEIGHTS of the same weights are deleted post-build.
  - Prologue: tiny memset + a couple of N=128 warmup matmuls start the HAM
    clock ramp right after the SPMD start barrier while the first weight/x
    DMAs (split across the three DMA-capable queues) land.
  - Tail: HL2+HL3 run fused per chunk-pair so early pairs' logits DMA out
    while later pairs still compute; the last (tiny) pair leaves only ~1us
    after the final matmul.
  - Host scatters per-core outputs back to original row order and applies
    sigmoid(logit + b3).
"""

import os
import sys

import ml_dtypes
import numpy as np

for _p in ("/opt/trn_rl_repo", "/root/.axon_site/_ro/trn_rl_repo"):
    if os.path.isdir(_p) and _p not in sys.path:
        sys.path.append(_p)

import concourse.bacc as bacc
import concourse.bass as bass
import concourse.tile as tile
from concourse import mybir
from concourse.bass import MemorySpace
from concourse.bass_utils import run_bass_kernel_spmd

F32 = mybir.dt.float32
BF16 = mybir.dt.bfloat16
F8 = mybir.dt.float8e4
AF = mybir.ActivationFunctionType
DR = mybir.MatmulPerfMode.DoubleRow
NPBF16 = ml_dtypes.bfloat16
NPF8 = ml_dtypes.float8_e4m3  # TRN fp8e4: max +-240, RNE

B, D_IN = 16384, 256
S1, S2 = 1024, 1024
H1, H2 = 512, 256
C = 4
NCORES = 8
CAP = 2080  # rows per core; seed-0 max shard is exactly 2080
W512 = int(os.environ.get("K_W512", "8"))
W256 = int(os.environ.get("K_W256", "20"))

_nc_cache = {}
_last_results = None


def _chunks(cap):
    """Row chunks of <=512 (PSUM bank): [(off, n), ...]."""
    assert cap % 32 == 0 and cap <= 2560
    out = []
    off = 0
    while off < cap:
        n = min(512, cap - off)
        out.append((off, n))
        off += n
    return out


def _dedup_ldweights(nc):
    """Remove back-to-back InstLdweights that reload identical weights.

    The rust add_instruction splits every matmul into LDWEIGHTS + MATMUL.
    With the weight-stationary loop order most loads are redundant; the PE
    keeps the stationary operand between matmuls. Any waits on a removed
    LDWEIGHTS are merged into the instruction that followed it.
    """
    removed = kept = 0
    for f in nc.m.functions:
        for blk in f.blocks:
            insts = list(blk.instructions)
            new = []
            last_key = None
            pending_waits = []
            for inst in insts:
                nm = type(inst).__name__
                if nm == "InstLdweights":
                    key = (repr(inst.ins[0]), inst.perf_mode,
                           inst.tile_position, inst.is_transpose)
                    si = inst.sync_info
                    has_upd = bool(si is not None and si.on_update)
                    if key == last_key and not has_upd:
                        if si is not None and si.on_wait:
                            pending_waits.extend(si.on_wait)
                        removed += 1
                        continue
                    last_key = key
                    kept += 1
                elif nm == "InstMatmult":
                    if pending_waits:
                        si = inst.sync_info
                        if si is None:
                            inst.sync_info = mybir.SyncInfo(
                                on_wait=list(pending_waits), on_update=[])
                        else:
                            si.on_wait = list(si.on_wait) + pending_waits
                        pending_waits = []
                # Other instruction kinds run on non-PE engines (or are
                # semaphore ops) and do not disturb the PE weight array, so
                # the cached key stays valid across them.
                new.append(inst)
            assert not pending_waits
            blk.instructions[:] = new
    return removed, kept


def _build(cap=CAP):
    """Build the single-core MLP program (SPMD across 8 cores)."""
    nc = bacc.Bacc("TRN2", target_bir_lowering=False, debug=False)

    def din(name, shape, dt=F8):
        return nc.dram_tensor(name, list(shape), dt, kind="ExternalInput").ap()

    chs = _chunks(cap)
    ncks = len(chs)
    # pairs of chunks sharing one 2-bank psum tile
    prs = [tuple(range(i, min(i + 2, ncks))) for i in range(0, ncks, 2)]

    xT = din("xT", [128, 2, cap])            # x rows, k-major
    w1 = din("w1", [128, 2, S1])
    w2 = din("w2", [128, 8, S2])
    hw1 = din("hw1", [128, 8, H1])
    hw2 = din("hw2", [128, 4, H2])
    hw3 = din("hw3", [128, 2, 16])           # w3 in col 0, zero-padded
    # biases packed into one tensor: [b1(8) | b2(8) | hb1(4) | hb2(2) | pad]
    cst = din("consts", [128, 23], F32)
    out = nc.dram_tensor("out", [1, cap], F32, kind="ExternalOutput").ap()

    ALU = mybir.AluOpType

    with tile.TileContext(nc) as tc:
        with tc.tile_pool(name="weights", bufs=1) as wp, \
             tc.tile_pool(name="acts", bufs=1) as ap_, \
             tc.tile_pool(name="outs", bufs=4) as op, \
             tc.tile_pool(name="psum", bufs=4, space=MemorySpace.PSUM) as pp:

            w1s = wp.tile([128, 2, S1], F8, tag="w1s")
            w2s = wp.tile([128, 8, S2], F8, tag="w2s")
            hw1s = wp.tile([128, 8, H1], F8, tag="hw1s")
            hw2s = wp.tile([128, 4, H2], F8, tag="hw2s")
            hw3s = wp.tile([128, 2, 16], F8, tag="hw3s")
            csts = wp.tile([128, 23], F32, tag="csts")
            b1s = csts[:, 0:8]
            b2s = csts[:, 8:16]
            hb1s = csts[:, 16:20]
            hb2s = csts[:, 20:22]

            # activations stay resident for all chunks (weight-stationary);
            # [128, ktiles, chunk, 512] with the ragged tail chunk padded
            xts = ap_.tile([128, 2, cap], F8, tag="xts")
            h1s = ap_.tile([128, 8, ncks, 512], F8, tag="h1s")
            h2s = ap_.tile([128, 8, ncks, 512], F8, tag="h2s")
            a1s = ap_.tile([128, 4, ncks, 512], F8, tag="a1s")
            a2s = ap_.tile([128, 2, ncks, 512], F8, tag="a2s")

            # PE warm-up: dependency-free matmuls ramp the HAM clock and
            # fill the ~11us DMA-completion window before real matmuls.
            # Two phases: big N=512 MMs for the bulk, then N=256 MMs so the
            # handoff to the (DMA-gated) first real matmul is fine-grained —
            # a short overshoot is far cheaper than a PE gap, which would
            # re-throttle HAM for ~7us.
            if W512 or W256:
                wut = wp.tile([128, 512], BF16, tag="wut")
                nc.vector.memset(wut[:], 0.0)
                wups = pp.tile([128, 2, 512], F32, tag="ps")
                for _ in range(W512):
                    nc.tensor.matmul(wups[:, 0, :], wut[:, 0:128],
                                     wut[:, :], start=True, stop=True)
                for _ in range(W256):
                    nc.tensor.matmul(wups[:, 0, 0:256], wut[:, 0:128],
                                     wut[:, 0:256], start=True, stop=True)

            # DMAs across the three DMA-capable queues (sync/scalar/gpsimd),
            # ordered so completion semaphores fire in the order the compute
            # stream consumes them (w1+x0 first, then x1..x3/csts staggered,
            # then w2/hw*).  Transfers only start ~4us after their
            # descriptor (ring round-trip), so the cascade matters.
            nc.sync.dma_start(out=w1s[:, 0, :], in_=w1[:, 0, :])
            nc.scalar.dma_start(out=xts[:, :, 0:512], in_=xT[:, :, 0:512])
            nc.gpsimd.dma_start(out=w1s[:, 1, :], in_=w1[:, 1, :])
            nc.sync.dma_start(out=csts[:], in_=cst[:])
            nc.sync.dma_start(out=xts[:, :, 512:1024], in_=xT[:, :, 512:1024])
            nc.scalar.dma_start(out=xts[:, :, 1024:1536],
                                in_=xT[:, :, 1024:1536])
            nc.gpsimd.dma_start(out=xts[:, :, 1536:cap], in_=xT[:, :, 1536:cap])
            for k in range(8):
                nc.sync.dma_start(out=w2s[:, k, :], in_=w2[:, k, :])
            for k in range(8):
                nc.gpsimd.dma_start(out=hw1s[:, k, :], in_=hw1[:, k, :])
            nc.gpsimd.dma_start(out=hw2s[:], in_=hw2[:])
            nc.gpsimd.dma_start(out=hw3s[:], in_=hw3[:])

            # Bias+relu epilogue.  Chunk pairs are split across ACT and DVE
            # (one chunk each) so the psum tile drains in half the time —
            # the next layer's first matmul reuses these psum banks, so
            # epilogue latency is on the PE critical path at layer
            # boundaries.  Singleton (tail) chunks alternate engines.
            epi_n = [0]

            def epilogue1(dst, src, bias_ap):
                if epi_n[0] % 2 == 0:
                    nc.scalar.activation(dst, src, AF.Relu, bias=bias_ap)
                else:
                    nc.vector.tensor_scalar(
                        dst, src, bias_ap, 0.0, ALU.add, ALU.max)
                epi_n[0] += 1

            def epilogue_pair(dst0, src0, dst1, src1, bias_ap):
                nc.scalar.activation(dst0, src0, AF.Relu, bias=bias_ap)
                nc.vector.tensor_scalar(
                    dst1, src1, bias_ap, 0.0, ALU.add, ALU.max)

            def layer(rhs, ktiles, wt, nm, bias, dst):
                """dst[:,m,c,:] = relu(sum_k wt[:,k,m]T @ rhs(kp,c) + b)"""
                npair = ktiles // 2
                for m in range(nm):
                    tt = [pp.tile([128, 2, 512], F32, tag="ps", name=f"t{j}")
                          for j in range(len(prs))]
                    for kp in range(npair):
                        lhs = wt[:, 2 * kp:2 * kp + 2,
                                 m * 128:(m + 1) * 128]
                        for c, (off, n) in enumerate(chs):
                            nc.tensor.matmul(
                                tt[c // 2][:, c % 2, :n], lhs, rhs(kp, c),
                                start=(kp == 0), stop=(kp == npair - 1),
                                perf_mode=DR)
                    for j, pr in enumerate(prs):
                        if len(pr) == 2 and chs[pr[0]][1] == chs[pr[1]][1]:
                            n = chs[pr[0]][1]
                            epilogue_pair(
                                dst[:, m, pr[0], :n], tt[j][:, 0, :n],
                                dst[:, m, pr[1], :n], tt[j][:, 1, :n],
                                bias[:, m:m + 1])
                        else:
                            for c2, c in enumerate(pr):
                                epilogue1(dst[:, m, c, :chs[c][1]],
                                          tt[j][:, c2, :chs[c][1]],
                                          bias[:, m:m + 1])

            def hslice(t):
                return lambda kp, c: t[:, 2 * kp:2 * kp + 2, c, :chs[c][1]]

            layer(lambda kp, c: xts[:, :, chs[c][0]:chs[c][0] + chs[c][1]],
                  2, w1s, 8, b1s, h1s)                 # L1: 256 -> 1024
            layer(hslice(h1s), 8, w2s, 8, b2s, h2s)    # L2: 1024 -> 1024
            layer(hslice(h2s), 8, hw1s, 4, hb1s, a1s)  # HL1: 1024 -> 512

            # HL2 (512 -> 256) + HL3 (256 -> 1) software-pipelined per chunk
            # pair: HL3 of pair j is emitted after HL2 of pair j+1 so the
            # in-order PE never stalls on pair-j epilogues, and early pairs'
            # logits DMA out while later pairs still compute.  The ragged
            # tail pair comes last, leaving a tiny post-matmul tail.
            cp_n = [0]

            def hl2_pair(j):
                pr = prs[j]
                for m in range(2):
                    tt = pp.tile([128, 2, 512], F32, tag="ps",
                                 name=f"t2_{j}_{m}")
                    for kp in range(2):
                        lhs = hw2s[:, 2 * kp:2 * kp + 2,
                                   m * 128:(m + 1) * 128]
                        for c2, c in enumerate(pr):
                            nc.tensor.matmul(
                                tt[:, c2, :chs[c][1]], lhs,
                                a1s[:, 2 * kp:2 * kp + 2, c, :chs[c][1]],
                                start=(kp == 0), stop=(kp == 1),
                                perf_mode=DR)
                    if len(pr) == 2 and chs[pr[0]][1] == chs[pr[1]][1]:
                        n = chs[pr[0]][1]
                        epilogue_pair(
                            a2s[:, m, pr[0], :n], tt[:, 0, :n],
                            a2s[:, m, pr[1], :n], tt[:, 1, :n],
                            hb2s[:, m:m + 1])
                    else:
                        for c2, c in enumerate(pr):
                            epilogue1(a2s[:, m, c, :chs[c][1]],
                                      tt[:, c2, :chs[c][1]],
                                      hb2s[:, m:m + 1])

            # logits land in one contiguous SBUF strip; one out-DMA per pair
            ots = op.tile([1, cap], F32, tag="ots")

            def hl3_pair(j):
                # HL3: 256 -> 1 logits (M=1); fp32 psum -> SBUF copies split
                # across ACT/DVE per chunk, one out-DMA per pair on sync;
                # bias+sigmoid happen host-side.
                pr = prs[j]
                psl = pp.tile([128, 2, 512], F32, tag="ps", name=f"t3_{j}")
                for c2, c in enumerate(pr):
                    nc.tensor.matmul(psl[0:1, c2, :chs[c][1]],
                                     hw3s[:, :, 0:1],
                                     a2s[:, :, c, :chs[c][1]],
                                     start=True, stop=True, perf_mode=DR)
                for c2, c in enumerate(pr):
                    off, n = chs[c]
                    if cp_n[0] % 2 == 0:
                        nc.scalar.activation(ots[:, off:off + n],
                                             psl[0:1, c2, :n], AF.Copy)
                    else:
                        nc.vector.tensor_scalar_add(ots[:, off:off + n],
                                                    psl[0:1, c2, :n], 0.0)
                    cp_n[0] += 1
                # pair 0 DMAs from scalar; the last two pairs merge into one
                # sync DMA so only a single descriptor sits after the final
                # matmul.
                if j == 0:
                    lo, hi = chs[pr[0]][0], chs[pr[-1]][0] + chs[pr[-1]][1]
                    nc.scalar.dma_start(out=out[:, lo:hi], in_=ots[:, lo:hi])
                elif j == len(prs) - 1:
                    lo = chs[prs[1][0]][0]
                    hi = chs[pr[-1]][0] + chs[pr[-1]][1]
                    nc.sync.dma_start(out=out[:, lo:hi], in_=ots[:, lo:hi])

            seq = []
            for j in range(len(prs)):
                seq.append(("hl2", j))
                if j >= 1:
                    seq.append(("hl3", j - 1))
            seq.append(("hl3", len(prs) - 1))
            for kind, j in seq:
                (hl2_pair if kind == "hl2" else hl3_pair)(j)

    _dedup_ldweights(nc)
    nc.compile()
    return nc


def _get_nc(cap=CAP):
    if cap not in _nc_cache:
        _nc_cache[cap] = _build(cap)
    return _nc_cache[cap]


def _q8(v):
    return np.clip(v, -240.0, 240.0).astype(NPF8)


def _tile_k(w, ktiles):
    """[K, M] -> [128, ktiles, M] fp8 with K = ktiles*128, K idx = k*128+p."""
    k, m = w.shape
    assert k == ktiles * 128
    return np.ascontiguousarray(
        _q8(w.reshape(ktiles, 128, m).transpose(1, 0, 2)))


def _tile_b(b):
    """[M] -> [128, M/128] f32; column m holds bias for m-tile m."""
    m = b.shape[0]
    return np.ascontiguousarray(b.reshape(m // 128, 128).T.astype(np.float32))


def _make_in_maps(inputs):
    x = np.asarray(inputs["x"], dtype=np.float32)
    ff = np.asarray(inputs["feature_flags"]).astype(np.int64)
    idx = ff[:, 0] * 2 + ff[:, 1]

    W1 = np.asarray(inputs["W1"], np.float32)
    b1 = np.asarray(inputs["b1"], np.float32)
    W2 = np.asarray(inputs["W2"], np.float32)
    b2 = np.asarray(inputs["b2"], np.float32)
    HW1 = np.asarray(inputs["HW1"], np.float32)
    Hb1 = np.asarray(inputs["Hb1"], np.float32)
    HW2 = np.asarray(inputs["HW2"], np.float32)
    Hb2 = np.asarray(inputs["Hb2"], np.float32)
    HW3 = np.asarray(inputs["HW3"], np.float32)

    # Row assignment: combo c -> cores 2c, 2c+1.
    row_sets = []
    for c in range(C):
        rows = np.nonzero(idx == c)[0]
        h = (len(rows) + 1) // 2
        row_sets.append(rows[:h])
        row_sets.append(rows[h:])
    max_shard = max(len(r) for r in row_sets)
    cap = max(CAP, -(-max_shard // 32) * 32)

    w1t = _tile_k(W1, 2)
    w2t = _tile_k(W2, 8)
    hw1t = [_tile_k(HW1[c], 8) for c in range(C)]
    hw2t = [_tile_k(HW2[c], 4) for c in range(C)]
    hw3t = []
    for c in range(C):
        t = np.zeros((128, 2, 16), NPF8)
        t[:, :, 0] = _q8(HW3[c][:, 0].reshape(2, 128).T)
        hw3t.append(t)
    cstt = []
    for c in range(C):
        cst = np.zeros((128, 23), np.float32)
        cst[:, 0:8] = _tile_b(b1)
        cst[:, 8:16] = _tile_b(b2)
        cst[:, 16:20] = _tile_b(Hb1[c])
        cst[:, 20:22] = _tile_b(Hb2[c])
        cstt.append(cst)

    in_maps = []
    for d, rows in enumerate(row_sets):
        c = d // 2
        n = len(rows)
        xt = np.zeros((128, 2, cap), NPF8)
        if n:
            xt[:, :, :n] = _q8(x[rows].T.reshape(2, 128, n).transpose(
                1, 0, 2))
        in_maps.append({
            "xT": xt,
            "w1": w1t, "w2": w2t,
            "hw1": hw1t[c], "hw2": hw2t[c], "hw3": hw3t[c],
            "consts": cstt[c],
        })

    return in_maps, row_sets, cap


def kernel(**inputs):
    global _last_results
    in_maps, row_sets, cap = _make_in_maps(inputs)
    nc = _get_nc(cap)
    res = run_bass_kernel_spmd(nc, in_maps, core_ids=list(range(NCORES)))
    _last_results = res

    Hb3 = np.asarray(inputs["Hb3"], np.float64)
    out = np.empty(B, np.float32)
    for d, rows in enumerate(row_sets):
        if len(rows):
            logits = res.results[d]["out"][0, :len(rows)].astype(np.float64)
            logits += Hb3[d // 2, 0]
            out[rows] = (1.0 / (1.0 + np.exp(-logits))).astype(np.float32)
    return out
